# revision 10
# baseline (speedup 1.0000x reference)
"""InterSliceAttention TRN2 kernel.

Reference computation (per batch element b):
    curr = f_curr[b] as [N, C] tokens (N = H*W = 1024, C = 512)
    neigh = [f_prev[b]; f_next[b]] as [2N, C]
    Q = curr @ Wq.T ; K = neigh @ Wk.T ; V = neigh @ Wv.T
    8-head attention (hd = 64), softmax over 2N keys
    out = LayerNorm(curr + attn_out @ Wo.T) * gamma + beta   (LN over C)

Sharding: data-parallel over batch. B = 8 batch elements -> 8 NeuronCores,
one element per core; weights replicated. Everything on-chip is kept
channels-first ([C_part, token_free]) so no activation transposes are needed:
  Qt = Wq @ Xc            (channels-first, via lhsT = Wq^T)
  scoresT = K_h @ Q_h^T   ([2N, N], key-major so matmuls chain w/o transpose)
  expT = exp(scoresT * scale)                       (ACT, PSUM -> SBUF)
  [AO_h^T; rowsum] = [V_h | 1]^T @ expT             (PSUM accumulate over key tiles)
  AO_h = AO_h^T * (1/rowsum)                        (softmax denominator)
  Y = LN_c(Xc + Wo @ AOt) channels-first, stats via ones-matmul over partitions
"""

import numpy as np

NUM_CORES = 8
B, C, H, W = 8, 512, 32, 32
N = H * W          # 1024 query tokens
N2 = 2 * N         # 2048 key tokens
HEADS = 8
HD = C // HEADS    # 64
SCALE = HD ** -0.5
LN_EPS = 1e-5
P = 128
CT = C // P        # 4 channel tiles
JT = N2 // P       # 16 key-token tiles
FREE = 512         # fp32 moving-operand limit per matmul
QC = N // FREE     # 2 query chunks

USE_F32R = True    # float32r matmuls: full-speed PE (1 cyc/row at N>=256)

_CACHE = {}


def _emit(ctx, tc, io):
    import concourse.bass as bass
    from concourse import mybir
    from concourse.masks import make_identity

    nc = tc.nc
    f32 = mybir.dt.float32
    f32r = mybir.dt.float32r
    Alu = mybir.AluOpType
    Act = mybir.ActivationFunctionType

    mdt = f32r if USE_F32R else f32    # dtype for matmul-feeding SBUF tiles

    def R(ap):
        return ap

    def F(ap):  # f32 view of a matmul-dtype tile, for DVE/ACT consumers
        return ap.bitcast(f32) if USE_F32R else ap

    xc_d, xp_d, xnx_d, w_d, gamma_d, beta_d, y_d = io

    # ---------------- pools ----------------
    persist = ctx.enter_context(tc.tile_pool(name="persist", bufs=1))
    ps_mm = ctx.enter_context(tc.tile_pool(name="ps_mm", bufs=2, space="PSUM"))
    ps_att = ctx.enter_context(tc.tile_pool(name="ps_att", bufs=2, space="PSUM"))

    ident = persist.tile([P, P], f32, tag="ident")
    make_identity(nc, ident)
    ones_col = persist.tile([P, 1], mdt, tag="ones")
    nc.vector.memset(F(ones_col[:]), 1.0)

    xc_sb = [persist.tile([P, N], mdt, tag=f"xc{i}", name=f"xc{i}") for i in range(CT)]
    # only Wo^T must outlive stage B; Q/K/V weight transposes live in stage A/B scope
    wT_o = [persist.tile([P, C], mdt, tag=f"wo{i}", name=f"wo{i}") for i in range(CT)]
    qt_sb = [persist.tile([P, N], mdt, tag=f"qt{i}", name=f"qt{i}") for i in range(CT)]
    kt_sb = [persist.tile([P, N2], mdt, tag=f"kt{i}", name=f"kt{i}") for i in range(CT)]
    v1_sb = [persist.tile([P, HEADS, HD + 1], mdt, tag=f"v1{j}", name=f"v1{j}") for j in range(JT)]
    aot_sb = [persist.tile([P, N], mdt, tag=f"aot{i}", name=f"aot{i}") for i in range(CT)]
    gamma_ct = [persist.tile([P, 1], f32, tag=f"g{i}", name=f"g{i}") for i in range(CT)]
    beta_ct = [persist.tile([P, 1], f32, tag=f"b{i}", name=f"b{i}") for i in range(CT)]

    for i in range(CT):
        nc.sync.dma_start(out=xc_sb[i][:], in_=xc_d[i * P:(i + 1) * P, :])
        nc.sync.dma_start(out=gamma_ct[i][:], in_=gamma_d[i * P:(i + 1) * P, :])
        nc.sync.dma_start(out=beta_ct[i][:], in_=beta_d[i * P:(i + 1) * P, :])

    # ---------------- stage A/B: weights transpose + QKV projections ----------
    with tc.tile_pool(name="stageAB", bufs=1) as ab_pool, \
         tc.tile_pool(name="wnat", bufs=2) as wnat_pool:
        # neighbor features [C, 2N]: prev tokens then next tokens
        xn_sb = [ab_pool.tile([P, N2], mdt, tag=f"xn{i}", name=f"xn{i}") for i in range(CT)]
        for i in range(CT):
            nc.sync.dma_start(out=xn_sb[i][:, 0:N], in_=xp_d[i * P:(i + 1) * P, :])
            nc.sync.dma_start(out=xn_sb[i][:, N:N2], in_=xnx_d[i * P:(i + 1) * P, :])
        wT = {k: [ab_pool.tile([P, C], mdt, tag=f"w{k}{i}", name=f"w{k}{i}")
                  for i in range(CT)] for k in ("q", "k", "v")}
        wT["o"] = wT_o

        # W^T into SBUF via PE transposes of 128x128 blocks.
        for k in ("q", "k", "v", "o"):
            for j in range(CT):           # row-block of W (= col-block of W^T)
                wnat = wnat_pool.tile([P, C], f32, tag="wnat")
                nc.sync.dma_start(out=wnat[:], in_=w_d[k][j * P:(j + 1) * P, :])
                for i in range(CT):       # col-block of W (= row-block of W^T)
                    pst = ps_mm.tile([P, P], f32, tag="mm")
                    nc.tensor.transpose(pst[:], wnat[:, i * P:(i + 1) * P], ident[:])
                    nc.vector.tensor_copy(out=wT[k][i][:, j * P:(j + 1) * P], in_=pst[:])

        # Qt[C,N] = Wq @ Xc : lhsT = WqT slice, rhs = Xc
        for mo in range(CT):
            for qc in range(QC):
                ps = ps_mm.tile([P, FREE], f32, tag="mm")
                for kt in range(CT):
                    nc.tensor.matmul(
                        ps[:],
                        R(wT["q"][kt][:, mo * P:(mo + 1) * P]),
                        R(xc_sb[kt][:, qc * FREE:(qc + 1) * FREE]),
                        start=(kt == 0), stop=(kt == CT - 1))
                nc.scalar.copy(out=qt_sb[mo][:, qc * FREE:(qc + 1) * FREE], in_=ps[:])

        # Kt[C,2N] = Wk @ Xn
        for mo in range(CT):
            for qc in range(N2 // FREE):
                ps = ps_mm.tile([P, FREE], f32, tag="mm")
                for kt in range(CT):
                    nc.tensor.matmul(
                        ps[:],
                        R(wT["k"][kt][:, mo * P:(mo + 1) * P]),
                        R(xn_sb[kt][:, qc * FREE:(qc + 1) * FREE]),
                        start=(kt == 0), stop=(kt == CT - 1))
                nc.scalar.copy(out=kt_sb[mo][:, qc * FREE:(qc + 1) * FREE], in_=ps[:])

        # V token-major [2N, C] = Xn^T @ Wv^T, packed into v1 = [V_h | 1] per head
        for j in range(JT):
            ps = ps_mm.tile([P, FREE], f32, tag="mm")
            for kt in range(CT):
                nc.tensor.matmul(
                    ps[:],
                    R(xn_sb[kt][:, j * P:(j + 1) * P]),
                    R(wT["v"][kt][:]),
                    start=(kt == 0), stop=(kt == CT - 1))
            nc.vector.memset(F(v1_sb[j][:, :, HD]), 1.0)
            nc.vector.tensor_copy(
                out=v1_sb[j][:, :, 0:HD],
                in_=ps[:].rearrange("p (h d) -> p h d", h=HEADS))

    # ---------------- stage C: attention per head ----------------
    with tc.tile_pool(name="stageC", bufs=1) as c_pool, \
         tc.tile_pool(name="expp", bufs=3) as exp_pool, \
         tc.tile_pool(name="tmpC", bufs=2) as tmpc_pool:
        for h in range(HEADS):
            hi, hr = h // 2, (h % 2) * HD
            ps_o = ps_att.tile([HD + 1, N], f32, tag="att")
            for j in range(JT):
                ps_s = ps_mm.tile([P, N], f32, tag="mm")
                for qc in range(QC):
                    nc.tensor.matmul(
                        ps_s[:, qc * FREE:(qc + 1) * FREE],
                        R(kt_sb[hi][hr:hr + HD, j * P:(j + 1) * P]),
                        R(qt_sb[hi][hr:hr + HD, qc * FREE:(qc + 1) * FREE]),
                        start=True, stop=True)
                e = exp_pool.tile([P, N], mdt, tag="exp")
                nc.scalar.activation(e[:], ps_s[:], Act.Exp, scale=SCALE)
                for qc in range(QC):
                    nc.tensor.matmul(
                        ps_o[:, qc * FREE:(qc + 1) * FREE],
                        R(v1_sb[j][:, h, :]),
                        R(e[:, qc * FREE:(qc + 1) * FREE]),
                        start=(j == 0), stop=(j == JT - 1))
            recip = tmpc_pool.tile([1, N], f32, tag="recip")
            nc.vector.reciprocal(recip[:], ps_o[HD:HD + 1, :])
            recip_b = tmpc_pool.tile([HD, N], f32, tag="recipb")
            nc.gpsimd.partition_broadcast(recip_b[:], recip[:])
            ao_tmp = tmpc_pool.tile([HD, N], mdt, tag="aotmp")
            nc.vector.tensor_mul(ao_tmp[:], ps_o[0:HD, :], recip_b[:])
            # partition-shifting copy into the packed channels-first AO tile
            nc.sync.dma_start(out=aot_sb[hi][hr:hr + HD, :], in_=ao_tmp[:])

    # ---------------- stage D: out_proj + residual + LayerNorm ----------------
    with tc.tile_pool(name="stageD", bufs=1) as d_pool, \
         tc.tile_pool(name="tmpD", bufs=2) as tmpd_pool:
        x_sb = [d_pool.tile([P, N], mdt, tag=f"x{i}", name=f"x{i}") for i in range(CT)]
        ps_s1 = ps_att.tile([1, N], f32, tag="att")
        ps_s2 = ps_att.tile([1, N], f32, tag="att")
        for ct in range(CT):
            ps_o = ps_mm.tile([P, N], f32, tag="mm")
            for qc in range(QC):
                for kt in range(CT):
                    nc.tensor.matmul(
                        ps_o[:, qc * FREE:(qc + 1) * FREE],
                        R(wT["o"][kt][:, ct * P:(ct + 1) * P]),
                        R(aot_sb[kt][:, qc * FREE:(qc + 1) * FREE]),
                        start=(kt == 0), stop=(kt == CT - 1))
            # x = proj + residual
            nc.vector.scalar_tensor_tensor(
                out=x_sb[ct][:], in0=ps_o[:], scalar=1.0, in1=F(xc_sb[ct][:]),
                op0=Alu.mult, op1=Alu.add)
            sq = tmpd_pool.tile([P, N], mdt, tag="sq")
            nc.vector.tensor_mul(sq[:], F(x_sb[ct][:]), F(x_sb[ct][:]))
            for qc in range(QC):
                nc.tensor.matmul(
                    ps_s1[:, qc * FREE:(qc + 1) * FREE],
                    R(ones_col[:]), R(x_sb[ct][:, qc * FREE:(qc + 1) * FREE]),
                    start=(ct == 0), stop=(ct == CT - 1))
                nc.tensor.matmul(
                    ps_s2[:, qc * FREE:(qc + 1) * FREE],
                    R(ones_col[:]), R(sq[:, qc * FREE:(qc + 1) * FREE]),
                    start=(ct == 0), stop=(ct == CT - 1))

        mu = d_pool.tile([1, N], f32, tag="mu")
        nc.vector.tensor_scalar_mul(mu[:], ps_s1[:], 1.0 / C)
        mu2 = d_pool.tile([1, N], f32, tag="mu2")
        nc.vector.tensor_mul(mu2[:], mu[:], mu[:])
        var = d_pool.tile([1, N], f32, tag="var")
        nc.vector.scalar_tensor_tensor(
            out=var[:], in0=ps_s2[:], scalar=1.0 / C, in1=mu2[:],
            op0=Alu.mult, op1=Alu.subtract)
        eps_t = d_pool.tile([1, 1], f32, tag="eps")
        nc.vector.memset(eps_t[:], LN_EPS)
        sd = d_pool.tile([1, N], f32, tag="sd")
        nc.scalar.activation(sd[:], var[:], Act.Sqrt, bias=eps_t[:])
        rinv = d_pool.tile([1, N], f32, tag="rinv")
        nc.vector.reciprocal(rinv[:], sd[:])
        mu_b = d_pool.tile([P, N], f32, tag="mub")
        nc.gpsimd.partition_broadcast(mu_b[:], mu[:])
        ri_b = d_pool.tile([P, N], f32, tag="rib")
        nc.gpsimd.partition_broadcast(ri_b[:], rinv[:])

        for ct in range(CT):
            t = tmpd_pool.tile([P, N], f32, tag="t")
            nc.vector.tensor_sub(t[:], F(x_sb[ct][:]), mu_b[:])
            nc.vector.tensor_mul(t[:], t[:], ri_b[:])
            y_sb = tmpd_pool.tile([P, N], f32, tag="y")
            nc.vector.tensor_scalar(
                out=y_sb[:], in0=t[:], scalar1=gamma_ct[ct][:],
                scalar2=beta_ct[ct][:], op0=Alu.mult, op1=Alu.add)
            nc.sync.dma_start(out=y_d[ct * P:(ct + 1) * P, :], in_=y_sb[:])


def _build():
    from contextlib import ExitStack

    import concourse.tile as tile
    from concourse import bacc, mybir

    f32 = mybir.dt.float32
    nc = bacc.Bacc("TRN2", target_bir_lowering=False, debug=False,
                   num_devices=NUM_CORES)
    feat_dt = mybir.dt.float32r if USE_F32R else f32
    xc_d = nc.dram_tensor("xc", [C, N], feat_dt, kind="ExternalInput").ap()
    xp_d = nc.dram_tensor("xp", [C, N], feat_dt, kind="ExternalInput").ap()
    xnx_d = nc.dram_tensor("xnx", [C, N], feat_dt, kind="ExternalInput").ap()
    w_d = {k: nc.dram_tensor(f"w{k}", [C, C], f32, kind="ExternalInput").ap()
           for k in ("q", "k", "v", "o")}
    gamma_d = nc.dram_tensor("gamma", [C, 1], f32, kind="ExternalInput").ap()
    beta_d = nc.dram_tensor("beta", [C, 1], f32, kind="ExternalInput").ap()
    y_d = nc.dram_tensor("y", [C, N], f32, kind="ExternalOutput").ap()

    with tile.TileContext(nc) as tc:
        with ExitStack() as ctx:
            _emit(ctx, tc, (xc_d, xp_d, xnx_d, w_d, gamma_d, beta_d, y_d))
    nc.compile()
    return nc


def _get_nc():
    if "nc" not in _CACHE:
        _CACHE["nc"] = _build()
    return _CACHE["nc"]


def _round_fp32r(a):
    """Round fp32 to the PE's fp32r format: RNE to 11 mantissa bits."""
    if not USE_F32R:
        return a
    u = np.ascontiguousarray(a).view(np.uint32).copy()
    lsb = (u >> 12) & np.uint32(1)
    u += np.uint32(0x7FF) + lsb
    u &= np.uint32(0xFFFFF000)
    return u.view(np.float32)


def make_in_maps(f_curr, f_prev, f_next, Wq, Wk, Wv, Wo, gamma, beta):
    f_curr = np.asarray(f_curr, dtype=np.float32).reshape(B, C, N)
    f_prev = np.asarray(f_prev, dtype=np.float32).reshape(B, C, N)
    f_next = np.asarray(f_next, dtype=np.float32).reshape(B, C, N)
    shared = {
        "wq": np.asarray(Wq, dtype=np.float32),
        "wk": np.asarray(Wk, dtype=np.float32),
        "wv": np.asarray(Wv, dtype=np.float32),
        "wo": np.asarray(Wo, dtype=np.float32),
        "gamma": np.asarray(gamma, dtype=np.float32).reshape(C, 1),
        "beta": np.asarray(beta, dtype=np.float32).reshape(C, 1),
    }
    return [
        {"xc": _round_fp32r(f_curr[b]), "xp": _round_fp32r(f_prev[b]),
         "xnx": _round_fp32r(f_next[b]), **shared}
        for b in range(NUM_CORES)
    ]


def kernel(f_curr, f_prev, f_next, Wq, Wk, Wv, Wo, gamma, beta):
    from concourse.bass_utils import run_bass_kernel_spmd

    nc = _get_nc()
    in_maps = make_in_maps(f_curr, f_prev, f_next, Wq, Wk, Wv, Wo, gamma, beta)
    res = run_bass_kernel_spmd(nc, in_maps, list(range(NUM_CORES)))
    out = np.stack([res.results[b]["y"] for b in range(NUM_CORES)])
    return out.reshape(B, C, H, W).astype(np.float32)


# revision 11
# speedup vs baseline: 39.2241x; 39.2241x over previous
"""InterSliceAttention TRN2 kernel.

Reference computation (per batch element b):
    curr = f_curr[b] as [N, C] tokens (N = H*W = 1024, C = 512)
    neigh = [f_prev[b]; f_next[b]] as [2N, C]
    Q = curr @ Wq.T ; K = neigh @ Wk.T ; V = neigh @ Wv.T
    8-head attention (hd = 64), softmax over 2N keys
    out = LayerNorm(curr + attn_out @ Wo.T) * gamma + beta   (LN over C)

Sharding: data-parallel over batch. B = 8 batch elements -> 8 NeuronCores,
one element per core; weights replicated. Everything on-chip is kept
channels-first ([C_part, token_free]) so no activation transposes are needed:
  Qt = Wq @ Xc            (channels-first, via lhsT = Wq^T)
  scoresT = K_h @ Q_h^T   ([2N, N], key-major so matmuls chain w/o transpose)
  expT = exp(scoresT * scale)                       (ACT, PSUM -> SBUF)
  [AO_h^T; rowsum] = [V_h | 1]^T @ expT             (PSUM accumulate over key tiles)
  AO_h = AO_h^T * (1/rowsum)                        (softmax denominator)
  Y = LN_c(Xc + Wo @ AOt) channels-first, stats via ones-matmul over partitions
"""

import numpy as np

NUM_CORES = 8
B, C, H, W = 8, 512, 32, 32
N = H * W          # 1024 query tokens
N2 = 2 * N         # 2048 key tokens
HEADS = 8
HD = C // HEADS    # 64
SCALE = HD ** -0.5
LN_EPS = 1e-5
P = 128
CT = C // P        # 4 channel tiles
JT = N2 // P       # 16 key-token tiles
FREE = 512         # fp32 moving-operand limit per matmul
QC = N // FREE     # 2 query chunks

USE_F32R = True    # float32r matmuls: full-speed PE (1 cyc/row at N>=256)

_CACHE = {}


def _emit(ctx, tc, io):
    import concourse.bass as bass
    from concourse import mybir
    from concourse.masks import make_identity

    nc = tc.nc
    f32 = mybir.dt.float32
    f32r = mybir.dt.float32r
    Alu = mybir.AluOpType
    Act = mybir.ActivationFunctionType

    mdt = f32r if USE_F32R else f32    # dtype for matmul-feeding SBUF tiles

    def R(ap):
        return ap

    def F(ap):  # f32 view of a matmul-dtype tile, for DVE/ACT consumers
        return ap.bitcast(f32) if USE_F32R else ap

    xc_d, xp_d, xnx_d, w_d, gamma_d, beta_d, y_d = io

    # ---------------- pools ----------------
    persist = ctx.enter_context(tc.tile_pool(name="persist", bufs=1))
    ps_mm = ctx.enter_context(tc.tile_pool(name="ps_mm", bufs=2, space="PSUM"))
    ps_att = ctx.enter_context(tc.tile_pool(name="ps_att", bufs=2, space="PSUM"))

    ident = persist.tile([P, P], f32, tag="ident")
    make_identity(nc, ident)
    ones_col = persist.tile([P, 1], mdt, tag="ones")
    nc.vector.memset(F(ones_col[:]), 1.0)

    xc_sb = [persist.tile([P, N], mdt, tag=f"xc{i}", name=f"xc{i}") for i in range(CT)]
    # only Wo^T must outlive stage B; Q/K/V weight transposes live in stage A/B scope
    wT_o = [persist.tile([P, C], mdt, tag=f"wo{i}", name=f"wo{i}") for i in range(CT)]
    qt_sb = [persist.tile([P, N], mdt, tag=f"qt{i}", name=f"qt{i}") for i in range(CT)]
    kt_sb = [persist.tile([P, N2], mdt, tag=f"kt{i}", name=f"kt{i}") for i in range(CT)]
    v1_sb = [persist.tile([P, HEADS, HD + 1], mdt, tag=f"v1{j}", name=f"v1{j}") for j in range(JT)]
    aot_sb = [persist.tile([P, N], mdt, tag=f"aot{i}", name=f"aot{i}") for i in range(CT)]
    gamma_ct = [persist.tile([P, 1], f32, tag=f"g{i}", name=f"g{i}") for i in range(CT)]
    beta_ct = [persist.tile([P, 1], f32, tag=f"b{i}", name=f"b{i}") for i in range(CT)]

    for i in range(CT):
        nc.sync.dma_start(out=xc_sb[i][:], in_=xc_d[i * P:(i + 1) * P, :])
        nc.sync.dma_start(out=gamma_ct[i][:], in_=gamma_d[i * P:(i + 1) * P, :])
        nc.sync.dma_start(out=beta_ct[i][:], in_=beta_d[i * P:(i + 1) * P, :])

    # ---------------- stage A/B: weights transpose + QKV projections ----------
    with tc.tile_pool(name="stageAB", bufs=1) as ab_pool, \
         tc.tile_pool(name="wnat", bufs=2) as wnat_pool:
        # neighbor features [C, 2N]: prev tokens then next tokens
        xn_sb = [ab_pool.tile([P, N2], mdt, tag=f"xn{i}", name=f"xn{i}") for i in range(CT)]
        for i in range(CT):
            nc.sync.dma_start(out=xn_sb[i][:, 0:N], in_=xp_d[i * P:(i + 1) * P, :])
            nc.sync.dma_start(out=xn_sb[i][:, N:N2], in_=xnx_d[i * P:(i + 1) * P, :])
        wT = {k: [ab_pool.tile([P, C], mdt, tag=f"w{k}{i}", name=f"w{k}{i}")
                  for i in range(CT)] for k in ("q", "k", "v")}
        wT["o"] = wT_o

        # W^T into SBUF via PE transposes of 128x128 blocks.
        for k in ("q", "k", "v", "o"):
            for j in range(CT):           # row-block of W (= col-block of W^T)
                wnat = wnat_pool.tile([P, C], f32, tag="wnat")
                nc.sync.dma_start(out=wnat[:], in_=w_d[k][j * P:(j + 1) * P, :])
                for i in range(CT):       # col-block of W (= row-block of W^T)
                    pst = ps_mm.tile([P, P], f32, tag="mm")
                    nc.tensor.transpose(pst[:], wnat[:, i * P:(i + 1) * P], ident[:])
                    nc.vector.tensor_copy(out=wT[k][i][:, j * P:(j + 1) * P], in_=pst[:])

        # Qt[C,N] = Wq @ Xc : lhsT = WqT slice, rhs = Xc
        for mo in range(CT):
            for qc in range(QC):
                ps = ps_mm.tile([P, FREE], f32, tag="mm")
                for kt in range(CT):
                    nc.tensor.matmul(
                        ps[:],
                        R(wT["q"][kt][:, mo * P:(mo + 1) * P]),
                        R(xc_sb[kt][:, qc * FREE:(qc + 1) * FREE]),
                        start=(kt == 0), stop=(kt == CT - 1))
                nc.scalar.copy(out=qt_sb[mo][:, qc * FREE:(qc + 1) * FREE], in_=ps[:])

        # Kt[C,2N] = Wk @ Xn
        for mo in range(CT):
            for qc in range(N2 // FREE):
                ps = ps_mm.tile([P, FREE], f32, tag="mm")
                for kt in range(CT):
                    nc.tensor.matmul(
                        ps[:],
                        R(wT["k"][kt][:, mo * P:(mo + 1) * P]),
                        R(xn_sb[kt][:, qc * FREE:(qc + 1) * FREE]),
                        start=(kt == 0), stop=(kt == CT - 1))
                nc.scalar.copy(out=kt_sb[mo][:, qc * FREE:(qc + 1) * FREE], in_=ps[:])

        # V token-major [2N, C] = Xn^T @ Wv^T, packed into v1 = [V_h | 1] per head
        for j in range(JT):
            ps = ps_mm.tile([P, FREE], f32, tag="mm")
            for kt in range(CT):
                nc.tensor.matmul(
                    ps[:],
                    R(xn_sb[kt][:, j * P:(j + 1) * P]),
                    R(wT["v"][kt][:]),
                    start=(kt == 0), stop=(kt == CT - 1))
            nc.vector.memset(F(v1_sb[j][:, :, HD]), 1.0)
            nc.vector.tensor_copy(
                out=v1_sb[j][:, :, 0:HD],
                in_=ps[:].rearrange("p (h d) -> p h d", h=HEADS))

    # ---------------- stage C: attention per head ----------------
    with tc.tile_pool(name="stageC", bufs=1) as c_pool, \
         tc.tile_pool(name="expp", bufs=3) as exp_pool, \
         tc.tile_pool(name="tmpC", bufs=2) as tmpc_pool:
        for h in range(HEADS):
            hi, hr = h // 2, (h % 2) * HD
            ps_o = ps_att.tile([HD + 1, N], f32, tag="att")
            for j in range(JT):
                ps_s = ps_mm.tile([P, N], f32, tag="mm")
                for qc in range(QC):
                    nc.tensor.matmul(
                        ps_s[:, qc * FREE:(qc + 1) * FREE],
                        R(kt_sb[hi][hr:hr + HD, j * P:(j + 1) * P]),
                        R(qt_sb[hi][hr:hr + HD, qc * FREE:(qc + 1) * FREE]),
                        start=True, stop=True)
                e = exp_pool.tile([P, N], mdt, tag="exp")
                nc.scalar.activation(e[:], ps_s[:], Act.Exp, scale=SCALE)
                for qc in range(QC):
                    nc.tensor.matmul(
                        ps_o[:, qc * FREE:(qc + 1) * FREE],
                        R(v1_sb[j][:, h, :]),
                        R(e[:, qc * FREE:(qc + 1) * FREE]),
                        start=(j == 0), stop=(j == JT - 1))
            recip = tmpc_pool.tile([1, N], f32, tag="recip")
            nc.vector.reciprocal(recip[:], ps_o[HD:HD + 1, :])
            recip_b = tmpc_pool.tile([HD, N], f32, tag="recipb")
            nc.gpsimd.partition_broadcast(recip_b[:], recip[:])
            ao_tmp = tmpc_pool.tile([HD, N], mdt, tag="aotmp")
            nc.vector.tensor_mul(ao_tmp[:], ps_o[0:HD, :], recip_b[:])
            # partition-shifting copy into the packed channels-first AO tile
            nc.sync.dma_start(out=aot_sb[hi][hr:hr + HD, :], in_=ao_tmp[:])

    # ---------------- stage D: out_proj + residual + LayerNorm ----------------
    with tc.tile_pool(name="stageD", bufs=1) as d_pool, \
         tc.tile_pool(name="tmpD", bufs=2) as tmpd_pool:
        x_sb = [d_pool.tile([P, N], mdt, tag=f"x{i}", name=f"x{i}") for i in range(CT)]
        ps_s1 = ps_att.tile([1, N], f32, tag="att")
        ps_s2 = ps_att.tile([1, N], f32, tag="att")
        for ct in range(CT):
            ps_o = ps_mm.tile([P, N], f32, tag="mm")
            for qc in range(QC):
                for kt in range(CT):
                    nc.tensor.matmul(
                        ps_o[:, qc * FREE:(qc + 1) * FREE],
                        R(wT["o"][kt][:, ct * P:(ct + 1) * P]),
                        R(aot_sb[kt][:, qc * FREE:(qc + 1) * FREE]),
                        start=(kt == 0), stop=(kt == CT - 1))
            # x = proj + residual
            nc.vector.scalar_tensor_tensor(
                out=x_sb[ct][:], in0=ps_o[:], scalar=1.0, in1=F(xc_sb[ct][:]),
                op0=Alu.mult, op1=Alu.add)
            sq = tmpd_pool.tile([P, N], mdt, tag="sq")
            nc.vector.tensor_mul(sq[:], F(x_sb[ct][:]), F(x_sb[ct][:]))
            for qc in range(QC):
                nc.tensor.matmul(
                    ps_s1[:, qc * FREE:(qc + 1) * FREE],
                    R(ones_col[:]), R(x_sb[ct][:, qc * FREE:(qc + 1) * FREE]),
                    start=(ct == 0), stop=(ct == CT - 1))
                nc.tensor.matmul(
                    ps_s2[:, qc * FREE:(qc + 1) * FREE],
                    R(ones_col[:]), R(sq[:, qc * FREE:(qc + 1) * FREE]),
                    start=(ct == 0), stop=(ct == CT - 1))

        mu = d_pool.tile([1, N], f32, tag="mu")
        nc.vector.tensor_scalar_mul(mu[:], ps_s1[:], 1.0 / C)
        mu2 = d_pool.tile([1, N], f32, tag="mu2")
        nc.vector.tensor_mul(mu2[:], mu[:], mu[:])
        var = d_pool.tile([1, N], f32, tag="var")
        nc.vector.scalar_tensor_tensor(
            out=var[:], in0=ps_s2[:], scalar=1.0 / C, in1=mu2[:],
            op0=Alu.mult, op1=Alu.subtract)
        eps_t = d_pool.tile([1, 1], f32, tag="eps")
        nc.vector.memset(eps_t[:], LN_EPS)
        sd = d_pool.tile([1, N], f32, tag="sd")
        nc.scalar.activation(sd[:], var[:], Act.Sqrt, bias=eps_t[:])
        rinv = d_pool.tile([1, N], f32, tag="rinv")
        nc.vector.reciprocal(rinv[:], sd[:])
        mu_b = d_pool.tile([P, N], f32, tag="mub")
        nc.gpsimd.partition_broadcast(mu_b[:], mu[:])
        ri_b = d_pool.tile([P, N], f32, tag="rib")
        nc.gpsimd.partition_broadcast(ri_b[:], rinv[:])

        for ct in range(CT):
            t = tmpd_pool.tile([P, N], f32, tag="t")
            nc.vector.tensor_sub(t[:], F(x_sb[ct][:]), mu_b[:])
            nc.vector.tensor_mul(t[:], t[:], ri_b[:])
            y_sb = tmpd_pool.tile([P, N], f32, tag="y")
            nc.vector.tensor_scalar(
                out=y_sb[:], in0=t[:], scalar1=gamma_ct[ct][:],
                scalar2=beta_ct[ct][:], op0=Alu.mult, op1=Alu.add)
            nc.sync.dma_start(out=y_d[ct * P:(ct + 1) * P, :], in_=y_sb[:])


def _build(reps=1):
    from contextlib import ExitStack

    import concourse.tile as tile
    from concourse import bacc, mybir

    f32 = mybir.dt.float32
    nc = bacc.Bacc("TRN2", target_bir_lowering=False, debug=False,
                   num_devices=NUM_CORES)
    feat_dt = mybir.dt.float32r if USE_F32R else f32
    xc_d = nc.dram_tensor("xc", [C, N], feat_dt, kind="ExternalInput").ap()
    xp_d = nc.dram_tensor("xp", [C, N], feat_dt, kind="ExternalInput").ap()
    xnx_d = nc.dram_tensor("xnx", [C, N], feat_dt, kind="ExternalInput").ap()
    w_d = {k: nc.dram_tensor(f"w{k}", [C, C], f32, kind="ExternalInput").ap()
           for k in ("q", "k", "v", "o")}
    gamma_d = nc.dram_tensor("gamma", [C, 1], f32, kind="ExternalInput").ap()
    beta_d = nc.dram_tensor("beta", [C, 1], f32, kind="ExternalInput").ap()
    y_d = nc.dram_tensor("y", [C, N], f32, kind="ExternalOutput").ap()

    with tile.TileContext(nc) as tc:
        for _ in range(reps):
            with ExitStack() as ctx:
                _emit(ctx, tc, (xc_d, xp_d, xnx_d, w_d, gamma_d, beta_d, y_d))
    nc.compile()
    return nc


def _get_nc(reps=1):
    key = ("nc", reps)
    if key not in _CACHE:
        _CACHE[key] = _build(reps)
    return _CACHE[key]


def _round_fp32r(a):
    """Round fp32 to the PE's fp32r format: RNE to 11 mantissa bits."""
    if not USE_F32R:
        return a
    u = np.ascontiguousarray(a).view(np.uint32).copy()
    lsb = (u >> 12) & np.uint32(1)
    u += np.uint32(0x7FF) + lsb
    u &= np.uint32(0xFFFFF000)
    return u.view(np.float32)


def make_in_maps(f_curr, f_prev, f_next, Wq, Wk, Wv, Wo, gamma, beta):
    f_curr = np.asarray(f_curr, dtype=np.float32).reshape(B, C, N)
    f_prev = np.asarray(f_prev, dtype=np.float32).reshape(B, C, N)
    f_next = np.asarray(f_next, dtype=np.float32).reshape(B, C, N)
    shared = {
        "wq": np.asarray(Wq, dtype=np.float32),
        "wk": np.asarray(Wk, dtype=np.float32),
        "wv": np.asarray(Wv, dtype=np.float32),
        "wo": np.asarray(Wo, dtype=np.float32),
        "gamma": np.asarray(gamma, dtype=np.float32).reshape(C, 1),
        "beta": np.asarray(beta, dtype=np.float32).reshape(C, 1),
    }
    return [
        {"xc": _round_fp32r(f_curr[b]), "xp": _round_fp32r(f_prev[b]),
         "xnx": _round_fp32r(f_next[b]), **shared}
        for b in range(NUM_CORES)
    ]


def kernel(f_curr, f_prev, f_next, Wq, Wk, Wv, Wo, gamma, beta):
    from concourse.bass_utils import run_bass_kernel_spmd

    nc = _get_nc()
    in_maps = make_in_maps(f_curr, f_prev, f_next, Wq, Wk, Wv, Wo, gamma, beta)
    res = run_bass_kernel_spmd(nc, in_maps, list(range(NUM_CORES)))
    out = np.stack([res.results[b]["y"] for b in range(NUM_CORES)])
    return out.reshape(B, C, H, W).astype(np.float32)
